# revision 9
# baseline (speedup 1.0000x reference)
"""Trainium2 Bass kernel for nn_Classifier (GNN edge-MLP link predictor).

Computes, for E candidate edges:
    out[e] = W2 . relu( x_nc[i0[e]] @ W1[:H] + x_pr[i1[e]] @ W1[H:] + b1 ) + b2

Strategy (8 NeuronCores, data-parallel over edges).  The expensive part of
this problem is the per-edge gather: dma_gather costs ~7.8ns of gpsimd
descriptor generation per gathered row, so the kernel eliminates half of
all gathers via a one-hot matmul trick and keeps everything SBUF-resident:

  - Precompute on device: A = x_nc @ W1[:H] + b1  and  B = x_pr @ W1[H:]
    (20000x128 each, bf16), stored node-major in SBUF as [128, 157*128]
    (node n lives at partition n%128, free block n//128).
  - Edges are bucketed (host-side) by pr-endpoint window w = i1 >> 7.
    Each core's 125k edges are laid out in 157 window segments of CAP
    slots (tail-padded with idx -1).
  - B-side "gather" per 512-edge tile: build a one-hot [128, 512] matrix
    on-chip (K=1 broadcast matmul of the window-local index row, then an
    is_equal against an iota column) and multiply by the window's 128-row
    slice of B.  Zero descriptors; exact row selection.
  - A-side gather: SBUF-source dma_gather per window (trailing -1 padding
    is skipped by the ucode; the true count comes from a runtime register
    so one SPMD program serves all 8 cores).
  - h = relu(psum_B + gA) on DVE+ScalarE, fc2 = [128,1] matmul, +b2 drain.

Host reorders per-edge outputs back from slot order (index bookkeeping
only; all FLOPs and data movement stay on-device).
"""

import numpy as np
import ml_dtypes

import concourse.bass as bass
import concourse.tile as tile
from concourse import bacc, mybir
from concourse import bass_utils

F32 = mybir.dt.float32
BF16 = mybir.dt.bfloat16
I16 = mybir.dt.int16
I32 = mybir.dt.int32

N_CORES = 8
H = 128
N_NODES = 20_000
NW = (N_NODES + 127) // 128  # 157 windows of 128 pr-nodes
E_TOTAL = 1_000_000
TILE = 512
SBUF_GATHER = False  # A-side gather source: SBUF-resident table vs HBM rows


def _build(cap: int):
    """Build + compile the SPMD program (cap = slots per window segment)."""
    assert cap % TILE == 0
    n_slots = NW * cap
    nt = cap // TILE  # tiles per window

    nc = bacc.Bacc(
        "TRN2", target_bir_lowering=False, debug=False, num_devices=N_CORES
    )

    xT = nc.dram_tensor("xT", [H, 2 * N_NODES], BF16, kind="ExternalInput").ap()
    w1a = nc.dram_tensor("w1a", [H, H], BF16, kind="ExternalInput").ap()
    w1b = nc.dram_tensor("w1b", [H, H], BF16, kind="ExternalInput").ap()
    b1b = nc.dram_tensor("b1b", [H, H], F32, kind="ExternalInput").ap()
    w2 = nc.dram_tensor("w2", [H, 1], BF16, kind="ExternalInput").ap()
    b2 = nc.dram_tensor("b2", [1, 1], F32, kind="ExternalInput").ap()
    iota = nc.dram_tensor("iota", [H, 1], F32, kind="ExternalInput").ap()
    ones = nc.dram_tensor("ones", [1, H], BF16, kind="ExternalInput").ap()
    idx0 = nc.dram_tensor("idx0", [16, n_slots // 16], I16, kind="ExternalInput").ap()
    A_dram = nc.dram_tensor("A_dram", [NW * 128, H], BF16, kind="Internal").ap()
    lidx = nc.dram_tensor("lidx", [NW, cap], BF16, kind="ExternalInput").ap()
    cnts = nc.dram_tensor("cnts", [1, NW], I32, kind="ExternalInput").ap()
    out = nc.dram_tensor("out", [1, n_slots], F32, kind="ExternalOutput").ap()

    relu = mybir.ActivationFunctionType.Relu
    ident = mybir.ActivationFunctionType.Identity
    add_op = mybir.AluOpType.add
    ieq = mybir.AluOpType.is_equal

    with tile.TileContext(nc) as tc:
        with (
            tc.tile_pool(name="const", bufs=1) as cpool,
            tc.tile_pool(name="x", bufs=2) as xpool,
            tc.tile_pool(name="g", bufs=3) as gpool,
            tc.tile_pool(name="oh", bufs=3) as ohpool,
            tc.tile_pool(name="h", bufs=3) as hpool,
            tc.tile_pool(name="l", bufs=2) as lpool,
            tc.tile_pool(name="st", bufs=2) as stpool,
            tc.tile_pool(name="pp", bufs=2, space="PSUM") as pppool,
            tc.tile_pool(name="bc", bufs=2, space="PSUM") as bcpool,
            tc.tile_pool(name="pb", bufs=2, space="PSUM") as pbpool,
            tc.tile_pool(name="p2", bufs=2, space="PSUM") as p2pool,
        ):
            # ---- constants ----
            w1a_sb = cpool.tile([H, H], BF16, tag="w1a")
            nc.sync.dma_start(w1a_sb[:], w1a[:])
            w1b_sb = cpool.tile([H, H], BF16, tag="w1b")
            nc.sync.dma_start(w1b_sb[:], w1b[:])
            b1b_sb = cpool.tile([H, H], F32, tag="b1b")
            nc.sync.dma_start(b1b_sb[:], b1b[:])
            w2_sb = cpool.tile([H, 1], BF16, tag="w2")
            nc.sync.dma_start(w2_sb[:], w2[:])
            b2_sb = cpool.tile([1, 1], F32, tag="b2")
            nc.sync.dma_start(b2_sb[:], b2[:])
            iota_sb = cpool.tile([H, 1], F32, tag="iota")
            nc.sync.dma_start(iota_sb[:], iota[:])
            ones_sb = cpool.tile([1, H], BF16, tag="ones")
            nc.sync.dma_start(ones_sb[:], ones[:])
            cnts_sb = cpool.tile([1, NW], I32, tag="cnts")
            nc.sync.dma_start(cnts_sb[:], cnts[:])
            idx0_sb = cpool.tile([128, n_slots // 16], I16, tag="idx0")
            for k in range(8):
                nc.sync.dma_start(idx0_sb[16 * k : 16 * (k + 1), :], idx0[:])

            # ---- node tables (node-major: node n -> partition n%128, block n//128)
            A_sb = cpool.tile([128, NW * H], BF16, tag="A")
            B_sb = cpool.tile([128, NW * H], BF16, tag="B")

            CH = 2048  # nodes per staged x chunk (16 blocks of 128)
            for tbl in range(2):  # 0: A (x_nc @ W1a + b1), 1: B (x_pr @ W1b)
                base = tbl * N_NODES
                wsb = w1a_sb if tbl == 0 else w1b_sb
                dst = A_sb if tbl == 0 else B_sb
                for off in range(0, N_NODES, CH):
                    cw = min(CH, N_NODES - off)
                    xc = xpool.tile([H, CH], BF16, tag="xt")
                    nc.sync.dma_start(xc[:, 0:cw], xT[:, base + off : base + off + cw])
                    for b in range(0, cw, 128):
                        c = (off + b) // 128
                        nn = min(128, cw - b)
                        ps = pppool.tile([128, H], F32, tag="pp")
                        nc.tensor.matmul(
                            ps[0:nn, :], xc[:, b : b + nn], wsb[:], start=True, stop=True
                        )
                        if tbl == 0:
                            if SBUF_GATHER:
                                nc.vector.tensor_tensor(
                                    dst[0:nn, H * c : H * (c + 1)],
                                    ps[0:nn, :],
                                    b1b_sb[0:nn, :],
                                    add_op,
                                )
                            else:
                                ast = hpool.tile([128, H], BF16, tag="adrain")
                                nc.vector.tensor_tensor(
                                    ast[0:nn, :], ps[0:nn, :], b1b_sb[0:nn, :], add_op
                                )
                                nc.sync.dma_start(
                                    A_dram[128 * c : 128 * c + nn, :], ast[0:nn, :]
                                )
                        else:
                            nc.scalar.activation(
                                dst[0:nn, H * c : H * (c + 1)], ps[0:nn, :], ident
                            )

            # ---- edge loop: one window (cap slots) at a time ----
            cnt_reg = nc.gpsimd.alloc_register("cnt_reg")
            for w in range(NW):
                kw = min(128, N_NODES - 128 * w)  # pr-nodes in this window
                nc.gpsimd.reg_load(cnt_reg, cnts_sb[0:1, w : w + 1])
                cnt_val = cnt_reg
                gA = gpool.tile([128, cap], BF16, tag="gA")
                if SBUF_GATHER:
                    nc.gpsimd.dma_gather(
                        gA[:].rearrange("p (one t) -> p one t", one=1),
                        A_sb[:],
                        idx0_sb[:, w * (cap // 16) : (w + 1) * (cap // 16)],
                        cap,
                        cnt_val,
                        H,
                        transpose=True,
                        single_packet=False,
                        sbuf_tokens_per_rank=128,
                        sbuf_free_dim_per_rank=2 * H,
                        sbuf_free_dim_pad_per_rank=0,
                        sbuf_byte_offset=0,
                    )
                else:
                    nc.gpsimd.dma_gather(
                        gA[:].rearrange("p (one t) -> p one t", one=1),
                        A_dram,
                        idx0_sb[:, w * (cap // 16) : (w + 1) * (cap // 16)],
                        cap,
                        cnt_val,
                        H,
                        transpose=True,
                        single_packet=False,
                    )
                lrow = lpool.tile([1, cap], BF16, tag="lidx")
                nc.sync.dma_start(lrow[:], lidx[w : w + 1, :])
                st = stpool.tile([1, cap], F32, tag="st")
                for t in range(nt):
                    sl = slice(t * TILE, (t + 1) * TILE)
                    bc = bcpool.tile([128, TILE], F32, tag="bc")
                    nc.tensor.matmul(
                        bc[:], ones_sb[:], lrow[:, sl], start=True, stop=True
                    )
                    oh = ohpool.tile([128, TILE], BF16, tag="oh")
                    nc.vector.tensor_scalar(oh[:], bc[:], iota_sb[:], None, ieq)
                    pb = pbpool.tile([128, TILE], F32, tag="pb")
                    nc.tensor.matmul(
                        pb[:],
                        B_sb[0:kw, H * w : H * (w + 1)],
                        oh[0:kw, :],
                        start=True,
                        stop=True,
                    )
                    hpre = hpool.tile([128, TILE], BF16, tag="hpre")
                    nc.vector.scalar_tensor_tensor(
                        hpre[:], pb[:], 0.0, gA[:, sl], add_op, add_op
                    )
                    h = hpool.tile([128, TILE], BF16, tag="h")
                    nc.scalar.activation(h[:], hpre[:], relu)
                    p2 = p2pool.tile([1, TILE], F32, tag="p2")
                    nc.tensor.matmul(p2[:], w2_sb[:], h[:], start=True, stop=True)
                    nc.vector.tensor_scalar(st[:, sl], p2[:], b2_sb[:], None, add_op)
                nc.sync.dma_start(out[:, w * cap : (w + 1) * cap], st[:])

    nc.compile()
    return nc


# ---------------------------------------------------------------------------
# Host-side wrapper
# ---------------------------------------------------------------------------

_CACHE: dict = {}


def _get_program(cap: int):
    if cap not in _CACHE:
        _CACHE[cap] = _build(cap)
    return _CACHE[cap]


def kernel(
    x_ncRNA: np.ndarray,
    x_Protein: np.ndarray,
    edge_label_index: np.ndarray,
    W1: np.ndarray,
    b1: np.ndarray,
    W2: np.ndarray,
    b2: np.ndarray,
    _trace: bool = False,
) -> np.ndarray:
    E = edge_label_index.shape[1]
    n_nodes = x_ncRNA.shape[0]
    assert n_nodes == N_NODES and x_Protein.shape[0] == N_NODES
    assert E % N_CORES == 0
    e_core = E // N_CORES

    # ---- shared (replicated) inputs ----
    xT = np.ascontiguousarray(
        np.concatenate([x_ncRNA.T, x_Protein.T], axis=1).astype(ml_dtypes.bfloat16)
    )
    w1a = np.ascontiguousarray(W1[:H].astype(ml_dtypes.bfloat16))
    w1b = np.ascontiguousarray(W1[H:].astype(ml_dtypes.bfloat16))
    b1b = np.ascontiguousarray(np.tile(b1.reshape(1, H), (H, 1)).astype(np.float32))
    w2 = np.ascontiguousarray(W2.reshape(H, 1).astype(ml_dtypes.bfloat16))
    b2_ = np.ascontiguousarray(b2.reshape(1, 1).astype(np.float32))
    iota = np.arange(H, dtype=np.float32).reshape(H, 1)
    ones = np.ones((1, H), dtype=ml_dtypes.bfloat16)

    # ---- per-core bucketing by pr-window ----
    ei = np.asarray(edge_label_index)
    percore = []
    cap = 1024
    for c in range(N_CORES):
        sl = slice(c * e_core, (c + 1) * e_core)
        i0 = ei[0, sl].astype(np.int64)
        i1 = ei[1, sl].astype(np.int64)
        w = (i1 >> 7).astype(np.int64)
        order = np.argsort(w, kind="stable")
        cnts = np.bincount(w, minlength=NW).astype(np.int32)
        cap = max(cap, TILE * int(np.ceil(cnts.max() / TILE)))
        percore.append((i0, i1, w, order, cnts))

    n_slots = NW * cap
    nc = _get_program(cap)

    in_maps = []
    unpack = []
    for c in range(N_CORES):
        i0, i1, w, order, cnts = percore[c]
        starts = np.zeros(NW, np.int64)
        starts[1:] = np.cumsum(cnts)[:-1]
        # slot of the k-th sorted edge: w*cap + (k - start_w)
        ws = w[order]
        slots = ws * cap + (np.arange(e_core) - starts[ws])
        idx0_slots = np.full(n_slots, -1, np.int16)
        idx0_slots[slots] = i0[order].astype(np.int16)
        lidx_slots = np.zeros(n_slots, np.float32)
        lidx_slots[slots] = (i1[order] - (ws << 7)).astype(np.float32)
        in_maps.append(
            {
                "xT": xT,
                "w1a": w1a,
                "w1b": w1b,
                "b1b": b1b,
                "w2": w2,
                "b2": b2_,
                "iota": iota,
                "ones": ones,
                "idx0": np.ascontiguousarray(
                    idx0_slots.reshape(n_slots // 16, 16).T
                ),
                "lidx": np.ascontiguousarray(
                    lidx_slots.reshape(NW, cap).astype(ml_dtypes.bfloat16)
                ),
                "cnts": np.ascontiguousarray(cnts.reshape(1, NW)),
            }
        )
        unpack.append((order, slots))

    res = bass_utils.run_bass_kernel_spmd(
        nc, in_maps, core_ids=list(range(N_CORES)), trace=_trace
    )
    out = np.empty(E, np.float32)
    for c in range(N_CORES):
        order, slots = unpack[c]
        o = res.results[c]["out"].reshape(-1)
        seg = out[c * e_core : (c + 1) * e_core]
        seg[order] = o[slots]
    kernel._last_results = res
    return out


# revision 10
# speedup vs baseline: 1.1344x; 1.1344x over previous
"""Trainium2 Bass kernel for nn_Classifier (GNN edge-MLP link predictor).

Computes, for E candidate edges:
    out[e] = W2 . relu( x_nc[i0[e]] @ W1[:H] + x_pr[i1[e]] @ W1[H:] + b1 ) + b2

Strategy (8 NeuronCores, data-parallel over edges).  The expensive part of
this problem is the per-edge gather: dma_gather costs ~7.8ns of gpsimd
descriptor generation per gathered row, so the kernel eliminates half of
all gathers via a one-hot matmul trick and keeps everything SBUF-resident:

  - Precompute on device: A = x_nc @ W1[:H] + b1  and  B = x_pr @ W1[H:]
    (20000x128 each, bf16), stored node-major in SBUF as [128, 157*128]
    (node n lives at partition n%128, free block n//128).
  - Edges are bucketed (host-side) by pr-endpoint window w = i1 >> 7.
    Each core's 125k edges are laid out in 157 window segments of CAP
    slots (tail-padded with idx -1).
  - B-side "gather" per 512-edge tile: build a one-hot [128, 512] matrix
    on-chip (K=1 broadcast matmul of the window-local index row, then an
    is_equal against an iota column) and multiply by the window's 128-row
    slice of B.  Zero descriptors; exact row selection.
  - A-side gather: SBUF-source dma_gather per window (trailing -1 padding
    is skipped by the ucode; the true count comes from a runtime register
    so one SPMD program serves all 8 cores).
  - h = relu(psum_B + gA) on DVE+ScalarE, fc2 = [128,1] matmul, +b2 drain.

Host reorders per-edge outputs back from slot order (index bookkeeping
only; all FLOPs and data movement stay on-device).
"""

import numpy as np
import ml_dtypes

import concourse.bass as bass
import concourse.tile as tile
from concourse import bacc, mybir
from concourse import bass_utils

F32 = mybir.dt.float32
BF16 = mybir.dt.bfloat16
I16 = mybir.dt.int16
I32 = mybir.dt.int32

N_CORES = 8
H = 128
N_NODES = 20_000
NW = (N_NODES + 127) // 128  # 157 windows of 128 pr-nodes
E_TOTAL = 1_000_000
TILE = 512
SBUF_GATHER = True  # A-side gather source: SBUF-resident table vs HBM rows


def _build(cap: int):
    """Build + compile the SPMD program (cap = slots per window segment)."""
    assert cap % TILE == 0
    n_slots = NW * cap
    nt = cap // TILE  # tiles per window

    nc = bacc.Bacc(
        "TRN2", target_bir_lowering=False, debug=False, num_devices=N_CORES
    )

    xT = nc.dram_tensor("xT", [H, 2 * N_NODES], BF16, kind="ExternalInput").ap()
    w1a = nc.dram_tensor("w1a", [H, H], BF16, kind="ExternalInput").ap()
    w1b = nc.dram_tensor("w1b", [H, H], BF16, kind="ExternalInput").ap()
    b1b = nc.dram_tensor("b1b", [H, H], F32, kind="ExternalInput").ap()
    w2 = nc.dram_tensor("w2", [H, 1], BF16, kind="ExternalInput").ap()
    b2 = nc.dram_tensor("b2", [1, 1], F32, kind="ExternalInput").ap()
    iota = nc.dram_tensor("iota", [H, 1], F32, kind="ExternalInput").ap()
    ones = nc.dram_tensor("ones", [1, H], BF16, kind="ExternalInput").ap()
    idx0 = nc.dram_tensor("idx0", [16, n_slots // 16], I16, kind="ExternalInput").ap()
    A_dram = nc.dram_tensor("A_dram", [NW * 128, H], BF16, kind="Internal").ap()
    lidx = nc.dram_tensor("lidx", [NW, cap], BF16, kind="ExternalInput").ap()
    cnts = nc.dram_tensor("cnts", [1, NW], I32, kind="ExternalInput").ap()
    out = nc.dram_tensor("out", [1, n_slots], F32, kind="ExternalOutput").ap()

    relu = mybir.ActivationFunctionType.Relu
    ident = mybir.ActivationFunctionType.Identity
    add_op = mybir.AluOpType.add
    ieq = mybir.AluOpType.is_equal

    with tile.TileContext(nc) as tc:
        with (
            tc.tile_pool(name="const", bufs=1) as cpool,
            tc.tile_pool(name="x", bufs=2) as xpool,
            tc.tile_pool(name="g", bufs=3) as gpool,
            tc.tile_pool(name="oh", bufs=3) as ohpool,
            tc.tile_pool(name="h", bufs=3) as hpool,
            tc.tile_pool(name="l", bufs=2) as lpool,
            tc.tile_pool(name="st", bufs=2) as stpool,
            tc.tile_pool(name="pp", bufs=2, space="PSUM") as pppool,
            tc.tile_pool(name="bc", bufs=2, space="PSUM") as bcpool,
            tc.tile_pool(name="pb", bufs=2, space="PSUM") as pbpool,
            tc.tile_pool(name="p2", bufs=2, space="PSUM") as p2pool,
        ):
            # ---- constants ----
            w1a_sb = cpool.tile([H, H], BF16, tag="w1a")
            nc.sync.dma_start(w1a_sb[:], w1a[:])
            w1b_sb = cpool.tile([H, H], BF16, tag="w1b")
            nc.sync.dma_start(w1b_sb[:], w1b[:])
            b1b_sb = cpool.tile([H, H], F32, tag="b1b")
            nc.sync.dma_start(b1b_sb[:], b1b[:])
            w2_sb = cpool.tile([H, 1], BF16, tag="w2")
            nc.sync.dma_start(w2_sb[:], w2[:])
            b2_sb = cpool.tile([1, 1], F32, tag="b2")
            nc.sync.dma_start(b2_sb[:], b2[:])
            iota_sb = cpool.tile([H, 1], F32, tag="iota")
            nc.sync.dma_start(iota_sb[:], iota[:])
            ones_sb = cpool.tile([1, H], BF16, tag="ones")
            nc.sync.dma_start(ones_sb[:], ones[:])
            cnts_sb = cpool.tile([1, NW], I32, tag="cnts")
            nc.sync.dma_start(cnts_sb[:], cnts[:])
            idx0_sb = cpool.tile([128, n_slots // 16], I16, tag="idx0")
            for k in range(8):
                nc.sync.dma_start(idx0_sb[16 * k : 16 * (k + 1), :], idx0[:])

            # ---- node tables (node-major: node n -> partition n%128, block n//128)
            A_sb = cpool.tile([128, NW * H], BF16, tag="A")
            B_sb = cpool.tile([128, NW * H], BF16, tag="B")

            CH = 2048  # nodes per staged x chunk (16 blocks of 128)
            for tbl in range(2):  # 0: A (x_nc @ W1a + b1), 1: B (x_pr @ W1b)
                base = tbl * N_NODES
                wsb = w1a_sb if tbl == 0 else w1b_sb
                dst = A_sb if tbl == 0 else B_sb
                for off in range(0, N_NODES, CH):
                    cw = min(CH, N_NODES - off)
                    xc = xpool.tile([H, CH], BF16, tag="xt")
                    nc.sync.dma_start(xc[:, 0:cw], xT[:, base + off : base + off + cw])
                    for b in range(0, cw, 128):
                        c = (off + b) // 128
                        nn = min(128, cw - b)
                        ps = pppool.tile([128, H], F32, tag="pp")
                        nc.tensor.matmul(
                            ps[0:nn, :], xc[:, b : b + nn], wsb[:], start=True, stop=True
                        )
                        if tbl == 0:
                            if SBUF_GATHER:
                                nc.vector.tensor_tensor(
                                    dst[0:nn, H * c : H * (c + 1)],
                                    ps[0:nn, :],
                                    b1b_sb[0:nn, :],
                                    add_op,
                                )
                            else:
                                ast = hpool.tile([128, H], BF16, tag="adrain")
                                nc.vector.tensor_tensor(
                                    ast[0:nn, :], ps[0:nn, :], b1b_sb[0:nn, :], add_op
                                )
                                nc.sync.dma_start(
                                    A_dram[128 * c : 128 * c + nn, :], ast[0:nn, :]
                                )
                        else:
                            nc.scalar.activation(
                                dst[0:nn, H * c : H * (c + 1)], ps[0:nn, :], ident
                            )

            # ---- edge loop: one window (cap slots) at a time ----
            for w in range(NW):
                kw = min(128, N_NODES - 128 * w)  # pr-nodes in this window
                cnt_val = cap
                gA = gpool.tile([128, cap], BF16, tag="gA")
                if SBUF_GATHER:
                    nc.gpsimd.dma_gather(
                        gA[:].rearrange("p (one t) -> p one t", one=1),
                        A_sb[:],
                        idx0_sb[:, w * (cap // 16) : (w + 1) * (cap // 16)],
                        cap,
                        cnt_val,
                        H,
                        transpose=True,
                        single_packet=False,
                        sbuf_tokens_per_rank=128,
                        sbuf_free_dim_per_rank=2 * H,
                        sbuf_free_dim_pad_per_rank=0,
                        sbuf_byte_offset=0,
                    )
                else:
                    nc.gpsimd.dma_gather(
                        gA[:].rearrange("p (one t) -> p one t", one=1),
                        A_dram,
                        idx0_sb[:, w * (cap // 16) : (w + 1) * (cap // 16)],
                        cap,
                        cnt_val,
                        H,
                        transpose=True,
                        single_packet=False,
                    )
                lrow = lpool.tile([1, cap], BF16, tag="lidx")
                nc.sync.dma_start(lrow[:], lidx[w : w + 1, :])
                st = stpool.tile([1, cap], F32, tag="st")
                for t in range(nt):
                    sl = slice(t * TILE, (t + 1) * TILE)
                    bc = bcpool.tile([128, TILE], F32, tag="bc")
                    nc.tensor.matmul(
                        bc[:], ones_sb[:], lrow[:, sl], start=True, stop=True
                    )
                    oh = ohpool.tile([128, TILE], BF16, tag="oh")
                    nc.vector.tensor_scalar(oh[:], bc[:], iota_sb[:], None, ieq)
                    pb = pbpool.tile([128, TILE], F32, tag="pb")
                    nc.tensor.matmul(
                        pb[:],
                        B_sb[0:kw, H * w : H * (w + 1)],
                        oh[0:kw, :],
                        start=True,
                        stop=True,
                    )
                    hpre = hpool.tile([128, TILE], BF16, tag="hpre")
                    nc.vector.scalar_tensor_tensor(
                        hpre[:], pb[:], 0.0, gA[:, sl], add_op, add_op
                    )
                    h = hpool.tile([128, TILE], BF16, tag="h")
                    nc.scalar.activation(h[:], hpre[:], relu)
                    p2 = p2pool.tile([1, TILE], F32, tag="p2")
                    nc.tensor.matmul(p2[:], w2_sb[:], h[:], start=True, stop=True)
                    nc.vector.tensor_scalar(st[:, sl], p2[:], b2_sb[:], None, add_op)
                nc.sync.dma_start(out[:, w * cap : (w + 1) * cap], st[:])

    nc.compile()
    return nc


# ---------------------------------------------------------------------------
# Host-side wrapper
# ---------------------------------------------------------------------------

_CACHE: dict = {}


def _get_program(cap: int):
    if cap not in _CACHE:
        _CACHE[cap] = _build(cap)
    return _CACHE[cap]


def kernel(
    x_ncRNA: np.ndarray,
    x_Protein: np.ndarray,
    edge_label_index: np.ndarray,
    W1: np.ndarray,
    b1: np.ndarray,
    W2: np.ndarray,
    b2: np.ndarray,
    _trace: bool = False,
) -> np.ndarray:
    E = edge_label_index.shape[1]
    n_nodes = x_ncRNA.shape[0]
    assert n_nodes == N_NODES and x_Protein.shape[0] == N_NODES
    assert E % N_CORES == 0
    e_core = E // N_CORES

    # ---- shared (replicated) inputs ----
    xT = np.ascontiguousarray(
        np.concatenate([x_ncRNA.T, x_Protein.T], axis=1).astype(ml_dtypes.bfloat16)
    )
    w1a = np.ascontiguousarray(W1[:H].astype(ml_dtypes.bfloat16))
    w1b = np.ascontiguousarray(W1[H:].astype(ml_dtypes.bfloat16))
    b1b = np.ascontiguousarray(np.tile(b1.reshape(1, H), (H, 1)).astype(np.float32))
    w2 = np.ascontiguousarray(W2.reshape(H, 1).astype(ml_dtypes.bfloat16))
    b2_ = np.ascontiguousarray(b2.reshape(1, 1).astype(np.float32))
    iota = np.arange(H, dtype=np.float32).reshape(H, 1)
    ones = np.ones((1, H), dtype=ml_dtypes.bfloat16)

    # ---- per-core bucketing by pr-window ----
    ei = np.asarray(edge_label_index)
    percore = []
    cap = 1024
    for c in range(N_CORES):
        sl = slice(c * e_core, (c + 1) * e_core)
        i0 = ei[0, sl].astype(np.int64)
        i1 = ei[1, sl].astype(np.int64)
        w = (i1 >> 7).astype(np.int64)
        order = np.argsort(w, kind="stable")
        cnts = np.bincount(w, minlength=NW).astype(np.int32)
        cap = max(cap, TILE * int(np.ceil(cnts.max() / TILE)))
        percore.append((i0, i1, w, order, cnts))

    n_slots = NW * cap
    nc = _get_program(cap)

    in_maps = []
    unpack = []
    for c in range(N_CORES):
        i0, i1, w, order, cnts = percore[c]
        starts = np.zeros(NW, np.int64)
        starts[1:] = np.cumsum(cnts)[:-1]
        # slot of the k-th sorted edge: w*cap + (k - start_w)
        ws = w[order]
        slots = ws * cap + (np.arange(e_core) - starts[ws])
        idx0_slots = np.zeros(n_slots, np.int16)
        idx0_slots[slots] = i0[order].astype(np.int16)
        lidx_slots = np.zeros(n_slots, np.float32)
        lidx_slots[slots] = (i1[order] - (ws << 7)).astype(np.float32)
        in_maps.append(
            {
                "xT": xT,
                "w1a": w1a,
                "w1b": w1b,
                "b1b": b1b,
                "w2": w2,
                "b2": b2_,
                "iota": iota,
                "ones": ones,
                "idx0": np.ascontiguousarray(
                    idx0_slots.reshape(n_slots // 16, 16).T
                ),
                "lidx": np.ascontiguousarray(
                    lidx_slots.reshape(NW, cap).astype(ml_dtypes.bfloat16)
                ),
                "cnts": np.ascontiguousarray(cnts.reshape(1, NW)),
            }
        )
        unpack.append((order, slots))

    res = bass_utils.run_bass_kernel_spmd(
        nc, in_maps, core_ids=list(range(N_CORES)), trace=_trace
    )
    out = np.empty(E, np.float32)
    for c in range(N_CORES):
        order, slots = unpack[c]
        o = res.results[c]["out"].reshape(-1)
        seg = out[c * e_core : (c + 1) * e_core]
        seg[order] = o[slots]
    kernel._last_results = res
    return out
